# revision 20
# baseline (speedup 1.0000x reference)
"""MoE LoRA adapter layer on 8 trn2 NeuronCores.

Data-parallel over tokens: the (B=4, L=2048) token grid is split into 8
shards of 1024 tokens, each entirely inside one batch element, so every
core only needs the gate of its own batch (computed on-device from the
cls token) and no collectives.

Per-core device program (tokens T=1024, H=1024, E*R=64):
  router:  logits = W_router @ cls  (DVE bcast-mul-reduce)
           top-2 masked softmax -> gate (1,8) -> per-partition scale (64,1)
  mm1:     lowT (64,T) = sum_k A[k].T @ xT[k]   (PE, f32r)
           xT produced by PE 128x128 transposes, evacuated PSUM->SBUF
  scale:   lowT_s = lowT * gate_scale           (fused with PSUM evac)
  mm2:     delta (128,H) per token tile = lowT_s.T @ B  (PE, f32r)
  out:     y = delta + x (DVE add during PSUM evac), DMA to DRAM
"""

import sys

import numpy as np

if "/opt/trn_rl_repo" not in sys.path:
    sys.path.insert(0, "/opt/trn_rl_repo")

B, L, H = 4, 2048, 1024
E, R = 8, 8
ER = E * R
NCORES = 8
TOK = (B * L) // NCORES  # tokens per core = 1024
P = 128
KT = H // P    # 8 contraction tiles
NT = TOK // P  # 8 token tiles

_CACHE = {}


def _build(mm_dt_name: str):
    import concourse.bacc as bacc
    import concourse.mybir as mybir
    import concourse.tile as tile
    from concourse.masks import make_identity

    fp32 = mybir.dt.float32
    mmdt = getattr(mybir.dt, mm_dt_name)
    Alu = mybir.AluOpType
    Act = mybir.ActivationFunctionType

    nc = bacc.Bacc("TRN2")

    x_d = nc.dram_tensor("x", [TOK, H], mmdt, kind="ExternalInput")
    cls_d = nc.dram_tensor("cls", [1, H], fp32, kind="ExternalInput")
    wrt_d = nc.dram_tensor("w_router_t", [H, E], fp32, kind="ExternalInput")
    a_d = nc.dram_tensor("a_all", [H, ER], mmdt, kind="ExternalInput")
    b_d = nc.dram_tensor("b_all", [ER, H], mmdt, kind="ExternalInput")
    y_d = nc.dram_tensor("y", [TOK, H], fp32, kind="ExternalOutput")

    with tile.TileContext(nc) as tc:
        with (
            tc.tile_pool(name="const", bufs=1) as cpool,
            tc.tile_pool(name="xin", bufs=1) as xpool,
            tc.tile_pool(name="xt", bufs=1) as xtpool,
            tc.tile_pool(name="rt", bufs=1) as rpool,
            tc.tile_pool(name="yout", bufs=3) as ypool,
            tc.tile_pool(name="ps_t", bufs=2, space="PSUM") as pst,
            tc.tile_pool(name="ps_low", bufs=1, space="PSUM") as pslow,
            tc.tile_pool(name="ps_y", bufs=2, space="PSUM") as psy,
        ):
            # ---- constants / weights ----
            ident_f = cpool.tile([P, P], fp32)
            make_identity(nc, ident_f)
            ident = cpool.tile([P, P], mmdt)
            nc.vector.tensor_copy(ident, ident_f)
            ident8 = cpool.tile([E, E], fp32)
            make_identity(nc, ident8)

            # maskMT[e, p] = 1.0 iff p // R == e  (gate -> per-partition scale, via PE)
            maskMT = cpool.tile([E, ER], fp32)
            nc.gpsimd.memset(maskMT, 1.0)
            # keep where p - R*e >= 0
            nc.gpsimd.affine_select(
                out=maskMT, in_=maskMT, compare_op=Alu.is_ge, fill=0.0,
                base=0, pattern=[[1, ER]], channel_multiplier=-R,
            )
            # keep where (R-1) + R*e - p >= 0   (i.e. p - R*e <= R-1)
            nc.gpsimd.affine_select(
                out=maskMT, in_=maskMT, compare_op=Alu.is_ge, fill=0.0,
                base=R - 1, pattern=[[-1, ER]], channel_multiplier=R,
            )

            a_sb = cpool.tile([P, KT, ER], mmdt)
            nc.sync.dma_start(a_sb, a_d.rearrange("(k p) r -> p k r", p=P))
            b_sb = cpool.tile([ER, H], mmdt)
            nc.sync.dma_start(b_sb, b_d[:, :])
            wrt_sb = cpool.tile([P, KT, E], fp32)
            nc.sync.dma_start(wrt_sb, wrt_d.rearrange("(k p) e -> p k e", p=P))
            cls_sb = cpool.tile([1, H], fp32)
            nc.sync.dma_start(cls_sb, cls_d[:, :])

            # ---- router: logits = cls @ W_router.T via PE ----
            # clsT[:, k] = cls[0, k*P:(k+1)*P] transposed to partitions
            clsT = rpool.tile([P, KT], fp32)
            ps_c = pst.tile([P, KT], fp32, tag="ps_tr")
            for k in range(KT):
                nc.tensor.transpose(
                    ps_c[:, k:k + 1], cls_sb[0:1, k * P:(k + 1) * P], ident8[:1, :1]
                )
            nc.vector.tensor_copy(clsT, ps_c)
            ps_l = pst.tile([1, E], fp32, tag="ps_tr")
            for k in range(KT):
                nc.tensor.matmul(
                    ps_l, clsT[:, k:k + 1], wrt_sb[:, k, :],
                    start=(k == 0), stop=(k == KT - 1),
                )
            logits = rpool.tile([1, E], fp32)
            nc.vector.tensor_copy(logits, ps_l)

            m1 = rpool.tile([1, 1], fp32)
            nc.vector.reduce_max(out=m1, in_=logits, axis=mybir.AxisListType.X)
            eq = rpool.tile([1, E], fp32)
            nc.vector.tensor_scalar(eq, logits, m1, None, Alu.is_equal)
            masked = rpool.tile([1, E], fp32)
            nc.vector.tensor_scalar_mul(masked, eq, -1e30)
            nc.vector.tensor_tensor(masked, masked, logits, Alu.add)
            m2 = rpool.tile([1, 1], fp32)
            nc.vector.reduce_max(out=m2, in_=masked, axis=mybir.AxisListType.X)

            neg_m1 = rpool.tile([1, 1], fp32)
            nc.vector.tensor_scalar_mul(neg_m1, m1, -1.0)
            expv = rpool.tile([1, E], fp32)
            nc.scalar.activation(expv, logits, Act.Exp, bias=neg_m1, scale=1.0)
            ge = rpool.tile([1, E], fp32)
            nc.vector.tensor_scalar(ge, logits, m2, None, Alu.is_ge)
            nc.vector.tensor_tensor(expv, expv, ge, Alu.mult)
            denom = rpool.tile([1, 1], fp32)
            nc.vector.reduce_sum(out=denom, in_=expv, axis=mybir.AxisListType.X)
            rinv = rpool.tile([1, 1], fp32)
            nc.vector.reciprocal(rinv, denom)
            gate = rpool.tile([1, E], fp32)
            nc.vector.tensor_scalar_mul(gate, expv, rinv)

            # gateT (E,1) then gate_scale (ER,1) = maskMT.T @ gateT via PE
            ps_gt = pst.tile([E, 1], fp32, tag="ps_tr")
            nc.tensor.transpose(ps_gt, gate, ident8[:1, :1])
            gateT = rpool.tile([E, 1], fp32)
            nc.vector.tensor_copy(gateT, ps_gt)
            ps_gs = pst.tile([ER, 1], fp32, tag="ps_tr")
            nc.tensor.matmul(ps_gs, maskMT, gateT, start=True, stop=True)
            gate_scale = rpool.tile([ER, 1], fp32)
            nc.vector.tensor_copy(gate_scale, ps_gs)

            # ---- pipelined over 4 token chunks (2 token-tiles each) ----
            x_sb = xpool.tile([P, NT, H], mmdt)
            xt_sb = xtpool.tile([P, KT, TOK], mmdt)
            ps_lowT = pslow.tile([ER, TOK], fp32)
            lowT_s = rpool.tile([ER, TOK], mmdt)
            x_r = x_d.rearrange("(n p) h -> p n h", p=P)
            y_r = y_d.rearrange("(n p) h -> p n h", p=P)
            NCHUNK = 4
            TPC = NT // NCHUNK      # token tiles per chunk = 2
            CW = TPC * P            # chunk width in tokens = 256
            for c in range(NCHUNK):
                for t in range(TPC):
                    n = c * TPC + t
                    nc.sync.dma_start(x_sb[:, n, :], x_r[:, n, :])
                    # transpose the (128, 1024) block into xT columns
                    for half in range(2):
                        ps_tr = pst.tile([P, 512], mmdt)
                        for i in range(4):
                            k = half * 4 + i
                            nc.tensor.transpose(
                                ps_tr[:, i * P:(i + 1) * P],
                                x_sb[:, n, k * P:(k + 1) * P],
                                ident,
                            )
                        dst = xt_sb[:, half * 4:(half + 1) * 4, n * P:(n + 1) * P]
                        src = ps_tr.rearrange("p (i c) -> p i c", c=P)
                        if (n + half) % 2 == 0:
                            nc.vector.tensor_copy(dst, src)
                        else:
                            nc.scalar.copy(dst, src)
                # mm1 for this chunk's token columns
                cs = c * CW
                for k in range(KT):
                    nc.tensor.matmul(
                        ps_lowT[:, cs:cs + CW],
                        a_sb[:, k, :],
                        xt_sb[:, k, cs:cs + CW],
                        start=(k == 0),
                        stop=(k == KT - 1),
                    )
                if c % 2 == 0:
                    nc.vector.tensor_scalar(
                        lowT_s[:, cs:cs + CW], ps_lowT[:, cs:cs + CW],
                        gate_scale, None, Alu.mult,
                    )
                else:
                    nc.scalar.activation(
                        lowT_s[:, cs:cs + CW], ps_lowT[:, cs:cs + CW],
                        Act.Copy, scale=gate_scale,
                    )
                # mm2 + residual + store for this chunk
                for t in range(TPC):
                    n = c * TPC + t
                    ps_out = psy.tile([P, H], fp32)
                    for half in range(2):
                        nc.tensor.matmul(
                            ps_out[:, half * 512:(half + 1) * 512],
                            lowT_s[:, n * P:(n + 1) * P],
                            b_sb[:, half * 512:(half + 1) * 512],
                            start=True,
                            stop=True,
                        )
                    y_sb = ypool.tile([P, H], fp32)
                    nc.vector.tensor_tensor(y_sb, ps_out, x_sb[:, n, :], Alu.add)
                    nc.sync.dma_start(y_r[:, n, :], y_sb)

    return nc


def _get_nc(mm_dt_name: str):
    if mm_dt_name not in _CACHE:
        nc = _build(mm_dt_name)
        if not nc.is_finalized():
            nc.finalize()
        _CACHE[mm_dt_name] = nc
    return _CACHE[mm_dt_name]


def kernel(x, W_router, A_down, B_up, mm_dt_name: str = "float32r", trace: bool = False):
    from concourse.bass_utils import run_bass_kernel_spmd

    x = np.ascontiguousarray(x, dtype=np.float32)
    W_router_t = np.ascontiguousarray(W_router.T, dtype=np.float32)
    A_all = np.ascontiguousarray(
        A_down.transpose(1, 0, 2).reshape(H, ER), dtype=np.float32
    )
    B_all = np.ascontiguousarray(B_up.reshape(ER, H), dtype=np.float32)

    nc = _get_nc(mm_dt_name)

    in_maps = []
    for c in range(NCORES):
        b = c // (NCORES // B)
        s = (c % (NCORES // B)) * TOK
        in_maps.append({
            "x": np.ascontiguousarray(x[b, s:s + TOK, :]),
            "cls": np.ascontiguousarray(x[b, 0:1, :]),
            "w_router_t": W_router_t,
            "a_all": A_all,
            "b_all": B_all,
        })

    res = run_bass_kernel_spmd(nc, in_maps, core_ids=list(range(NCORES)), trace=trace)

    y = np.empty((B, L, H), dtype=np.float32)
    for c in range(NCORES):
        b = c // (NCORES // B)
        s = (c % (NCORES // B)) * TOK
        y[b, s:s + TOK, :] = res.results[c]["y"]
    if trace:
        return y, res
    return y
